# revision 11
# baseline (speedup 1.0000x reference)
"""ARIG user-encoder Trainium2 kernel (8-core pure data parallel).

B=4096, T=200, D=128. Each core handles 512 batches, processed as 4 chunks
of 128 (partition = batch). Weighted reductions over hist are DVE
broadcast-multiplies (bf16) + contiguous pairwise tree-folds (bf16 2x mode).
The last-K window is fetched with one indirect-DMA gather of the 5
contiguous rows ending at cnt. Tiny matmuls run on PE with host-prefolded
weights:
  qk = mean_hist @ (Wq.T @ Wk) * 1/sqrt(D)   (scores = hist . qk + log decay)
  long_term = wh @ Wv.T,  wh = sum_t attn*hist
"""

import sys

sys.path.insert(0, "/opt/trn_rl_repo")

import numpy as np

import concourse.bass as bass
import concourse.bacc as bacc
import concourse.tile as tile
from concourse import mybir
from concourse.bass_utils import run_bass_kernel_spmd
from concourse.masks import make_identity

B, T, D = 4096, 200, 128
KSHORT = 5
NCORES = 8
BL = B // NCORES          # 512 batches per core
CHUNK = 128               # batches per chunk (partition dim)
NCHUNK = BL // CHUNK      # 4
TSUB = 50                 # t subtile
NTSUB = T // TSUB         # 4

F32 = mybir.dt.float32
BF16 = mybir.dt.bfloat16
U8 = mybir.dt.uint8
I32 = mybir.dt.int32
AX = mybir.AxisListType
ALU = mybir.AluOpType
ACT = mybir.ActivationFunctionType

_CACHE = {}


def _softplus(x):
    return np.log1p(np.exp(-abs(x))) + max(x, 0.0)


def _bc(ap, n, where):
    """Insert a 0-stride broadcast dim of size n into a 2D [p, f] AP.
    where='mid' -> [p, n, f]; where='inner' -> [p, f, n]."""
    if where == "mid":
        dims = [ap.ap[0], [0, n], ap.ap[1]]
    else:
        dims = [ap.ap[0], ap.ap[1], [0, n]]
    return bass.AP(tensor=ap.tensor, offset=ap.offset, ap=dims)


def _fold_t(nc, scr, tlen, out, tmp_pool):
    """Sum scr[:, 0:tlen, :] over axis t by contiguous pairwise folds.
    scr is [128, T?, D] bf16 (destroyed). Result added... written to out
    ([128, D] f32) by the final fold."""
    # fold down by halves (in place), odd handled by folding the tail
    cur = tlen
    while cur > 2:
        half = cur // 2
        rem = cur - 2 * half  # 0 or 1
        # scr[:, 0:half] += scr[:, half:2*half]
        nc.vector.tensor_add(scr[:, 0:half, :], scr[:, 0:half, :],
                             scr[:, half:2 * half, :])
        if rem:
            # fold the leftover slice into position 0
            nc.vector.tensor_add(scr[:, 0:1, :], scr[:, 0:1, :],
                                 scr[:, cur - 1:cur, :])
        cur = half
    if cur == 2:
        nc.vector.tensor_add(out, scr[:, 0, :], scr[:, 1, :])
    else:
        nc.vector.tensor_copy(out, scr[:, 0, :])



def _fold_t_gp(nc, scr, tlen, out):
    """_fold_t on the gpsimd engine."""
    cur = tlen
    while cur > 2:
        half = cur // 2
        rem = cur - 2 * half
        nc.gpsimd.tensor_tensor(scr[:, 0:half, :], scr[:, 0:half, :],
                                scr[:, half:2 * half, :], op=ALU.add)
        if rem:
            nc.gpsimd.tensor_tensor(scr[:, 0:1, :], scr[:, 0:1, :],
                                    scr[:, cur - 1:cur, :], op=ALU.add)
        cur = half
    if cur == 2:
        nc.gpsimd.tensor_tensor(out, scr[:, 0, :], scr[:, 1, :], op=ALU.add)
    else:
        nc.gpsimd.tensor_copy(out, scr[:, 0, :])

def _fold_d(nc, scr, tlen, out):
    """Sum scr[:, 0:tlen, 0:128] over inner d by contiguous pairwise folds;
    writes out [128, tlen] f32."""
    cur = D
    while cur > 2:
        half = cur // 2
        nc.vector.tensor_add(scr[:, 0:tlen, 0:half], scr[:, 0:tlen, 0:half],
                             scr[:, 0:tlen, half:2 * half])
        cur = half
    nc.vector.tensor_add(out, scr[:, 0:tlen, 0], scr[:, 0:tlen, 1])


def _build(alpha, gw0, gw1, gb):
    nc = bacc.Bacc("TRN2")

    hist = nc.declare_dram_parameter("hist", [BL, T, D], F32, isOutput=False)
    mask = nc.declare_dram_parameter("mask", [BL, T], U8, isOutput=False)
    age = nc.declare_dram_parameter("age", [BL, T], F32, isOutput=False)
    pop = nc.declare_dram_parameter("pop", [BL, T], F32, isOutput=False)
    w2 = nc.declare_dram_parameter("w2", [D, D], F32, isOutput=False)      # Wq.T@Wk/sqrt(D)
    wvt = nc.declare_dram_parameter("wvt", [D, D], F32, isOutput=False)    # Wv.T
    gam = nc.declare_dram_parameter("gam", [CHUNK, D], F32, isOutput=False)  # gamma bcast
    bet = nc.declare_dram_parameter("bet", [CHUNK, D], F32, isOutput=False)  # beta bcast
    iot = nc.declare_dram_parameter("iot", [CHUNK, T], F32, isOutput=False)  # arange(T) bcast
    rowb = nc.declare_dram_parameter("rowb", [CHUNK, NCHUNK], F32, isOutput=False)  # (b0+p)*T
    iotk = nc.declare_dram_parameter("iotk", [CHUNK, KSHORT], F32, isOutput=False)
    out = nc.declare_dram_parameter("out", [BL, D], F32, isOutput=True)
    hist_flat = hist.rearrange("b t d -> (b t) d")

    with tile.TileContext(nc) as tc:
        with (
            tc.tile_pool(name="hist", bufs=2) as hist_pool,
            tc.tile_pool(name="big", bufs=1) as big_pool,
            tc.tile_pool(name="scr2", bufs=2) as s2_pool,
            tc.tile_pool(name="small", bufs=1) as sm_pool,
            tc.tile_pool(name="dmain", bufs=2) as dm_pool,
            tc.tile_pool(name="tiny", bufs=3) as tn_pool,
            tc.tile_pool(name="const", bufs=1) as c_pool,
            tc.tile_pool(name="psum", bufs=4, space="PSUM") as ps_pool,
        ):
            # constants
            w2_sb = c_pool.tile([D, D], F32)
            nc.sync.dma_start(out=w2_sb, in_=w2[:, :])
            wvt_sb = c_pool.tile([D, D], F32)
            nc.sync.dma_start(out=wvt_sb, in_=wvt[:, :])
            gam_sb = c_pool.tile([CHUNK, D], F32)
            nc.sync.dma_start(out=gam_sb, in_=gam[:, :])
            bet_sb = c_pool.tile([CHUNK, D], F32)
            nc.sync.dma_start(out=bet_sb, in_=bet[:, :])
            iot_sb = c_pool.tile([CHUNK, T], F32)
            nc.sync.dma_start(out=iot_sb, in_=iot[:, :])
            rowb_sb = c_pool.tile([CHUNK, NCHUNK], F32)
            nc.sync.dma_start(out=rowb_sb, in_=rowb[:, :])
            iotk_sb = c_pool.tile([CHUNK, KSHORT], F32)
            nc.sync.dma_start(out=iotk_sb, in_=iotk[:, :])
            ident = c_pool.tile([CHUNK, CHUNK], F32)
            make_identity(nc, ident)
            zero_c = c_pool.tile([CHUNK, 1], F32)
            nc.vector.memset(zero_c, 0.0)
            tiny_c = c_pool.tile([CHUNK, 1], F32)
            nc.vector.memset(tiny_c, 1e-12)
            tc.strict_bb_all_engine_barrier()

            for c in range(NCHUNK):
                b0 = c * CHUNK
                # ---- loads (hist cast f32->bf16 via SWDGE) ----
                h_bf = hist_pool.tile([CHUNK, T, D], BF16)
                for s in range(NTSUB):
                    nc.gpsimd.dma_start(
                        out=h_bf[:, s * TSUB:(s + 1) * TSUB, :],
                        in_=hist[b0:b0 + CHUNK, s * TSUB:(s + 1) * TSUB, :],
                    )
                mask_u8 = dm_pool.tile([CHUNK, T], U8, tag="mask_u8")
                nc.sync.dma_start(out=mask_u8, in_=mask[b0:b0 + CHUNK, :])
                age_f = dm_pool.tile([CHUNK, T], F32, tag="age")
                nc.sync.dma_start(out=age_f, in_=age[b0:b0 + CHUNK, :])
                pop_f = dm_pool.tile([CHUNK, T], F32, tag="pop")
                nc.sync.dma_start(out=pop_f, in_=pop[b0:b0 + CHUNK, :])

                # ---- small prep ----
                maskf = sm_pool.tile([CHUNK, T], F32, tag="maskf")
                nc.vector.tensor_copy(maskf, mask_u8)
                msum = tn_pool.tile([CHUNK, 1], F32, tag="msum")
                nc.vector.reduce_sum(msum, maskf, axis=AX.X)
                mden = tn_pool.tile([CHUNK, 1], F32, tag="mden")
                nc.vector.tensor_scalar_add(mden, msum, 1e-6)
                mrec = tn_pool.tile([CHUNK, 1], F32, tag="mrec")
                nc.vector.reciprocal(mrec, mden)

                # decay log-bias: dl = log(exp(-alpha*age) + 1e-12)
                edec = sm_pool.tile([CHUNK, T], F32, tag="edec")
                nc.scalar.activation(edec, age_f, ACT.Exp, bias=zero_c, scale=-alpha)
                dl = sm_pool.tile([CHUNK, T], F32, tag="dl")
                nc.scalar.activation(dl, edec, ACT.Ln, bias=tiny_c)

                # ---- last-K gather: rows [s0, s0+5) with s0 = max(cnt-5, 0) ----
                cnt = tn_pool.tile([CHUNK, 1], F32, tag="cnt")
                nc.vector.tensor_scalar_max(cnt, msum, 1.0)
                s0 = tn_pool.tile([CHUNK, 1], F32, tag="s0")
                nc.vector.tensor_scalar(s0, cnt, -float(KSHORT), 0.0,
                                        op0=ALU.add, op1=ALU.max)
                gidx_f = tn_pool.tile([CHUNK, 1], F32, tag="gidx_f")
                nc.vector.tensor_add(gidx_f, s0, rowb_sb[:, c:c + 1])
                gidx = tn_pool.tile([CHUNK, 1], I32, tag="gidx")
                nc.vector.tensor_copy(gidx, gidx_f)
                gath = sm_pool.tile([CHUNK, KSHORT, D], F32, tag="gath")
                nc.gpsimd.indirect_dma_start(
                    out=gath.rearrange("p k d -> p (k d)"),
                    out_offset=None,
                    in_=hist_flat,
                    in_offset=bass.IndirectOffsetOnAxis(ap=gidx, axis=0),
                )

                # ---- P1: mean = fold_t(maskf*hist) / (msum+1e-6) ----
                macc = sm_pool.tile([CHUNK, D], F32, tag="macc")
                for h in range(2):
                    th = T // 2
                    scr = big_pool.tile([CHUNK, T // 2, D], BF16, tag="p1scr")
                    for tt in range(th):
                        nc.vector.tensor_scalar_mul(
                            scr[:, tt, :], h_bf[:, h * th + tt, :],
                            maskf[:, h * th + tt:h * th + tt + 1])
                    hpart = sm_pool.tile([CHUNK, D], F32, tag="hpart")
                    _fold_t(nc, scr, th, hpart, tn_pool)
                    if h == 0:
                        nc.vector.tensor_copy(macc, hpart)
                    else:
                        nc.vector.tensor_add(macc, macc, hpart)
                mean_sb = sm_pool.tile([CHUNK, D], F32, tag="mean")
                nc.vector.tensor_scalar_mul(mean_sb, macc, mrec)

                # ---- qk = mean @ W2 (PE) ----
                meanT_ps = ps_pool.tile([D, CHUNK], F32, tag="tp")
                nc.tensor.transpose(meanT_ps, mean_sb, ident)
                meanT_sb = sm_pool.tile([D, CHUNK], F32, tag="meanT")
                nc.scalar.copy(meanT_sb, meanT_ps)
                qk_ps = ps_pool.tile([CHUNK, D], F32, tag="mm")
                nc.tensor.matmul(qk_ps, meanT_sb, w2_sb, start=True, stop=True)
                qk_bf = sm_pool.tile([CHUNK, D], BF16, tag="qk")
                nc.scalar.copy(qk_bf, qk_ps)

                # ---- P2: scores = fold_d(qk*hist) + dl, mask -> softmax ----
                scores = sm_pool.tile([CHUNK, T], F32, tag="scores")
                for s in range(NTSUB):
                    t0 = s * TSUB
                    stmp = s2_pool.tile([CHUNK, TSUB, D], BF16, tag="scr2")
                    nc.vector.tensor_mul(
                        stmp, h_bf[:, t0:t0 + TSUB, :], _bc(qk_bf, TSUB, "mid"))
                    _fold_d(nc, stmp, TSUB, scores[:, t0:t0 + TSUB])
                nc.vector.tensor_add(scores, scores, dl)
                smask = sm_pool.tile([CHUNK, T], F32, tag="smask")
                nc.vector.memset(smask, -1e30)
                nc.vector.copy_predicated(smask, mask_u8, scores)
                smax = tn_pool.tile([CHUNK, 1], F32, tag="smax")
                nc.vector.tensor_reduce(smax, smask, axis=AX.X, op=ALU.max,
                                        negate=True)
                esc = sm_pool.tile([CHUNK, T], F32, tag="esc")
                ssum = tn_pool.tile([CHUNK, 1], F32, tag="ssum")
                nc.scalar.activation(esc, smask, ACT.Exp, bias=smax,
                                     accum_out=ssum)
                srec = tn_pool.tile([CHUNK, 1], F32, tag="srec")
                nc.vector.reciprocal(srec, ssum)

                # ---- P3: wh = fold_t(esc*hist) * srec ; long = wh @ Wv.T ----
                wacc = sm_pool.tile([CHUNK, D], F32, tag="wacc")
                for h in range(4):
                    th = T // 4
                    scr3 = s2_pool.tile([CHUNK, T // 4, D], BF16, tag="p3scr")
                    for tt in range(th):
                        nc.vector.tensor_scalar_mul(
                            scr3[:, tt, :], h_bf[:, h * th + tt, :],
                            esc[:, h * th + tt:h * th + tt + 1])
                    hpart = sm_pool.tile([CHUNK, D], F32, tag="whpart")
                    _fold_t(nc, scr3, th, hpart, tn_pool)
                    if h == 0:
                        nc.vector.tensor_copy(wacc, hpart)
                    else:
                        nc.vector.tensor_add(wacc, wacc, hpart)
                wh_sb = sm_pool.tile([CHUNK, D], F32, tag="wh")
                nc.vector.tensor_scalar_mul(wh_sb, wacc, srec)
                whT_ps = ps_pool.tile([D, CHUNK], F32, tag="tp")
                nc.tensor.transpose(whT_ps, wh_sb, ident)
                whT_sb = sm_pool.tile([D, CHUNK], F32, tag="whT")
                nc.scalar.copy(whT_sb, whT_ps)
                long_ps = ps_pool.tile([CHUNK, D], F32, tag="mm")
                nc.tensor.matmul(long_ps, whT_sb, wvt_sb, start=True, stop=True)
                long_sb = sm_pool.tile([CHUNK, D], F32, tag="long")
                nc.scalar.copy(long_sb, long_ps)

                # ---- short term from gathered rows ----
                denom = tn_pool.tile([CHUNK, 1], F32, tag="denom")
                nc.vector.tensor_scalar_min(denom, cnt, float(KSHORT))
                drec = tn_pool.tile([CHUNK, 1], F32, tag="drec")
                nc.vector.reciprocal(drec, denom)
                sacc = sm_pool.tile([CHUNK, D], F32, tag="sacc")
                cs = tn_pool.tile([CHUNK, 1], F32, tag="cs")
                nc.vector.tensor_tensor(cs, cnt, s0, op=ALU.subtract)
                wj5 = tn_pool.tile([CHUNK, KSHORT], F32, tag="wj5")
                nc.vector.tensor_scalar(wj5, iotk_sb, cs, None, op0=ALU.is_lt)
                gj = sm_pool.tile([CHUNK, D], F32, tag="gj")
                for j in range(KSHORT):
                    nc.vector.tensor_scalar_mul(gj, gath[:, j, :],
                                                wj5[:, j:j + 1])
                    if j == 0:
                        nc.vector.tensor_copy(sacc, gj)
                    else:
                        nc.vector.tensor_add(sacc, sacc, gj)
                short_sb = sm_pool.tile([CHUNK, D], F32, tag="short")
                nc.vector.tensor_scalar_mul(short_sb, sacc, drec)

                # ---- window means of pop/age -> gate ----
                lkg = sm_pool.tile([CHUNK, T], F32, tag="lkg")
                nc.vector.tensor_scalar(lkg, iot_sb, s0, None, op0=ALU.is_ge)
                lkl = sm_pool.tile([CHUNK, T], F32, tag="lkl")
                nc.vector.tensor_scalar(lkl, iot_sb, cnt, None, op0=ALU.is_lt)
                lk = sm_pool.tile([CHUNK, T], F32, tag="lk")
                nc.vector.tensor_mul(lk, lkg, lkl)
                lp = sm_pool.tile([CHUNK, T], F32, tag="lp")
                nc.vector.tensor_mul(lp, lk, pop_f)
                mp = tn_pool.tile([CHUNK, 1], F32, tag="mp")
                nc.vector.reduce_sum(mp, lp, axis=AX.X)
                nc.vector.tensor_mul(lp, lk, age_f)
                mr = tn_pool.tile([CHUNK, 1], F32, tag="mr")
                nc.vector.reduce_sum(mr, lp, axis=AX.X)
                z1 = tn_pool.tile([CHUNK, 1], F32, tag="z1")
                nc.vector.tensor_scalar_mul(z1, mp, gw0)
                z2 = tn_pool.tile([CHUNK, 1], F32, tag="z2")
                nc.vector.tensor_scalar_mul(z2, mr, gw1)
                nc.vector.tensor_add(z1, z1, z2)
                nc.vector.tensor_scalar_mul(z1, z1, drec)
                nc.vector.tensor_scalar_add(z1, z1, gb)
                ez = tn_pool.tile([CHUNK, 1], F32, tag="ez")
                nc.scalar.activation(ez, z1, ACT.Exp, bias=zero_c, scale=-1.0)
                ez1 = tn_pool.tile([CHUNK, 1], F32, tag="ez1")
                nc.vector.tensor_scalar_add(ez1, ez, 1.0)
                g = tn_pool.tile([CHUNK, 1], F32, tag="g")
                nc.vector.reciprocal(g, ez1)
                omg = tn_pool.tile([CHUNK, 1], F32, tag="omg")
                nc.vector.tensor_mul(omg, ez, g)

                # ---- combine + layernorm ----
                user = sm_pool.tile([CHUNK, D], F32, tag="user")
                nc.vector.tensor_scalar_mul(user, short_sb, g)
                ulong = sm_pool.tile([CHUNK, D], F32, tag="ulong")
                nc.vector.tensor_scalar_mul(ulong, long_sb, omg)
                nc.vector.tensor_add(user, user, ulong)

                stats = tn_pool.tile([CHUNK, 6], F32, tag="stats")
                nc.vector.bn_stats(stats, user)
                mv = tn_pool.tile([CHUNK, 2], F32, tag="mv")
                nc.vector.bn_aggr(mv, stats)
                veps = tn_pool.tile([CHUNK, 1], F32, tag="veps")
                nc.vector.tensor_scalar_add(veps, mv[:, 1:2], 1e-5)
                vrec = tn_pool.tile([CHUNK, 1], F32, tag="vrec")
                nc.vector.reciprocal(vrec, veps)  # 1/(var+eps)
                lnv = tn_pool.tile([CHUNK, 1], F32, tag="lnv")
                nc.scalar.activation(lnv, vrec, ACT.Ln, bias=tiny_c)
                rstd = tn_pool.tile([CHUNK, 1], F32, tag="rstd")
                nc.scalar.activation(rstd, lnv, ACT.Exp, bias=zero_c, scale=0.5)
                negmur = tn_pool.tile([CHUNK, 1], F32, tag="negmur")
                nc.vector.tensor_scalar(negmur, mv[:, 0:1], -1.0, rstd,
                                        op0=ALU.mult, op1=ALU.mult)
                usern = sm_pool.tile([CHUNK, D], F32, tag="usern")
                nc.scalar.activation(usern, user, ACT.Identity, bias=negmur,
                                     scale=rstd)
                nc.vector.tensor_mul(usern, usern, gam_sb)
                ou = sm_pool.tile([CHUNK, D], F32, tag="ou")
                nc.vector.tensor_add(ou, usern, bet_sb)
                nc.sync.dma_start(out=out[b0:b0 + CHUNK, :], in_=ou)

    nc.finalize()
    return nc


def _get_nc(alpha, gw0, gw1, gb):
    key = (round(alpha, 10), round(gw0, 10), round(gw1, 10), round(gb, 10))
    if key not in _CACHE:
        _CACHE[key] = _build(alpha, gw0, gw1, gb)
    return _CACHE[key]


def _run(inputs, trace=False):
    hist = np.ascontiguousarray(inputs["hist_items"], dtype=np.float32)
    mask = np.ascontiguousarray(inputs["hist_mask"]).astype(np.uint8)
    age = np.ascontiguousarray(inputs["hist_age_hours"], dtype=np.float32)
    pop = np.ascontiguousarray(inputs["hist_popularity"], dtype=np.float32)
    Wq = np.asarray(inputs["Wq"], dtype=np.float32)
    Wk = np.asarray(inputs["Wk"], dtype=np.float32)
    Wv = np.asarray(inputs["Wv"], dtype=np.float32)
    gate_w = np.asarray(inputs["gate_w"], dtype=np.float32)
    gate_b = np.asarray(inputs["gate_b"], dtype=np.float32)
    ln_gamma = np.asarray(inputs["ln_gamma"], dtype=np.float32)
    ln_beta = np.asarray(inputs["ln_beta"], dtype=np.float32)
    decay_alpha = float(np.asarray(inputs["decay_alpha"]))

    alpha = _softplus(decay_alpha) + 1e-6
    gw0, gw1 = float(gate_w[0, 0]), float(gate_w[0, 1])
    gb = float(gate_b[0])
    w2 = (Wq.T @ Wk) / np.sqrt(D)
    wvt = np.ascontiguousarray(Wv.T)
    gam = np.broadcast_to(ln_gamma, (CHUNK, D)).copy()
    bet = np.broadcast_to(ln_beta, (CHUNK, D)).copy()
    iot = np.broadcast_to(np.arange(T, dtype=np.float32), (CHUNK, T)).copy()
    rowb = np.empty((CHUNK, NCHUNK), np.float32)
    for c in range(NCHUNK):
        rowb[:, c] = (c * CHUNK + np.arange(CHUNK)) * T
    iotk = np.broadcast_to(np.arange(KSHORT, dtype=np.float32),
                           (CHUNK, KSHORT)).copy()

    nc = _get_nc(alpha, gw0, gw1, gb)
    in_maps = []
    for i in range(NCORES):
        sl = slice(i * BL, (i + 1) * BL)
        in_maps.append({
            "hist": hist[sl], "mask": mask[sl], "age": age[sl], "pop": pop[sl],
            "w2": w2, "wvt": wvt, "gam": gam, "bet": bet, "iot": iot,
            "rowb": rowb, "iotk": iotk,
        })
    res = run_bass_kernel_spmd(nc, in_maps, core_ids=list(range(NCORES)),
                               trace=trace)
    outs = [res.results[i]["out"] for i in range(NCORES)]
    full = np.concatenate(outs, axis=0).astype(np.float32)
    return full, res


def kernel(**inputs):
    return _run(inputs)[0]


# revision 13
# speedup vs baseline: 1.3218x; 1.3218x over previous
"""ARIG user-encoder Trainium2 kernel (8-core pure data parallel).

B=4096, T=200, D=128. Each core handles 512 batches, processed as 4 chunks
of 128 (partition = batch). Weighted reductions over hist are DVE
broadcast-multiplies (bf16) + contiguous pairwise tree-folds (bf16 2x mode).
The last-K window is fetched with one indirect-DMA gather of the 5
contiguous rows ending at cnt. Tiny matmuls run on PE with host-prefolded
weights:
  qk = mean_hist @ (Wq.T @ Wk) * 1/sqrt(D)   (scores = hist . qk + log decay)
  long_term = wh @ Wv.T,  wh = sum_t attn*hist
"""

import sys

sys.path.insert(0, "/opt/trn_rl_repo")

import numpy as np

import concourse.bass as bass
import concourse.bacc as bacc
import concourse.tile as tile
from concourse import mybir
from concourse.bass_utils import run_bass_kernel_spmd
from concourse.masks import make_identity

B, T, D = 4096, 200, 128
KSHORT = 5
NCORES = 8
BL = B // NCORES          # 512 batches per core
CHUNK = 128               # batches per chunk (partition dim)
NCHUNK = BL // CHUNK      # 4
TSUB = 50                 # t subtile
NTSUB = T // TSUB         # 4

F32 = mybir.dt.float32
BF16 = mybir.dt.bfloat16
U8 = mybir.dt.uint8
I32 = mybir.dt.int32
AX = mybir.AxisListType
ALU = mybir.AluOpType
ACT = mybir.ActivationFunctionType

_CACHE = {}


def _softplus(x):
    return np.log1p(np.exp(-abs(x))) + max(x, 0.0)


def _bc(ap, n, where):
    """Insert a 0-stride broadcast dim of size n into a 2D [p, f] AP.
    where='mid' -> [p, n, f]; where='inner' -> [p, f, n]."""
    if where == "mid":
        dims = [ap.ap[0], [0, n], ap.ap[1]]
    else:
        dims = [ap.ap[0], ap.ap[1], [0, n]]
    return bass.AP(tensor=ap.tensor, offset=ap.offset, ap=dims)



def _bc_pair(ap2, toff, tlen):
    """AP over a duplicated-weights tile w2[p, T, 2] (w duplicated along last
    axis) shaped [p, tlen, D//2, 2] with stride-0 on the D//2 dim and step-1
    innermost pair -> eligible for DVE 2x packing."""
    p = ap2.ap[0]
    return bass.AP(tensor=ap2.tensor, offset=ap2.offset + toff * 2,
                   ap=[p, [2, tlen], [0, D // 2], [1, 2]])


def _pairs(ap3, toff, tlen):
    """View h/scr tile AP [p, T?, D] as [p, tlen, D//2, 2] starting at toff."""
    p = ap3.ap[0]
    return bass.AP(tensor=ap3.tensor, offset=ap3.offset + toff * D,
                   ap=[p, [D, tlen], [2, D // 2], [1, 2]])

def _fold_t(nc, scr, tlen, out, tmp_pool):
    """Sum scr[:, 0:tlen, :] over axis t by contiguous pairwise folds.
    scr is [128, T?, D] bf16 (destroyed). Result added... written to out
    ([128, D] f32) by the final fold."""
    # fold down by halves (in place), odd handled by folding the tail
    cur = tlen
    while cur > 2:
        half = cur // 2
        rem = cur - 2 * half  # 0 or 1
        # scr[:, 0:half] += scr[:, half:2*half]
        nc.vector.tensor_add(scr[:, 0:half, :], scr[:, 0:half, :],
                             scr[:, half:2 * half, :])
        if rem:
            # fold the leftover slice into position 0
            nc.vector.tensor_add(scr[:, 0:1, :], scr[:, 0:1, :],
                                 scr[:, cur - 1:cur, :])
        cur = half
    if cur == 2:
        nc.vector.tensor_add(out, scr[:, 0, :], scr[:, 1, :])
    else:
        nc.vector.tensor_copy(out, scr[:, 0, :])



def _fold_t_gp(nc, scr, tlen, out):
    """_fold_t on the gpsimd engine."""
    cur = tlen
    while cur > 2:
        half = cur // 2
        rem = cur - 2 * half
        nc.gpsimd.tensor_tensor(scr[:, 0:half, :], scr[:, 0:half, :],
                                scr[:, half:2 * half, :], op=ALU.add)
        if rem:
            nc.gpsimd.tensor_tensor(scr[:, 0:1, :], scr[:, 0:1, :],
                                    scr[:, cur - 1:cur, :], op=ALU.add)
        cur = half
    if cur == 2:
        nc.gpsimd.tensor_tensor(out, scr[:, 0, :], scr[:, 1, :], op=ALU.add)
    else:
        nc.gpsimd.tensor_copy(out, scr[:, 0, :])

def _fold_d(nc, scr, tlen, out):
    """Sum scr[:, 0:tlen, 0:128] over inner d by contiguous pairwise folds;
    writes out [128, tlen] f32."""
    cur = D
    while cur > 2:
        half = cur // 2
        nc.vector.tensor_add(scr[:, 0:tlen, 0:half], scr[:, 0:tlen, 0:half],
                             scr[:, 0:tlen, half:2 * half])
        cur = half
    nc.vector.tensor_add(out, scr[:, 0:tlen, 0], scr[:, 0:tlen, 1])


def _build(alpha, gw0, gw1, gb):
    nc = bacc.Bacc("TRN2")

    hist = nc.declare_dram_parameter("hist", [BL, T, D], F32, isOutput=False)
    mask = nc.declare_dram_parameter("mask", [BL, T], U8, isOutput=False)
    age = nc.declare_dram_parameter("age", [BL, T], F32, isOutput=False)
    pop = nc.declare_dram_parameter("pop", [BL, T], F32, isOutput=False)
    w2 = nc.declare_dram_parameter("w2", [D, D], F32, isOutput=False)      # Wq.T@Wk/sqrt(D)
    wvt = nc.declare_dram_parameter("wvt", [D, D], F32, isOutput=False)    # Wv.T
    gam = nc.declare_dram_parameter("gam", [CHUNK, D], F32, isOutput=False)  # gamma bcast
    bet = nc.declare_dram_parameter("bet", [CHUNK, D], F32, isOutput=False)  # beta bcast
    iot = nc.declare_dram_parameter("iot", [CHUNK, T], F32, isOutput=False)  # arange(T) bcast
    rowb = nc.declare_dram_parameter("rowb", [CHUNK, NCHUNK], F32, isOutput=False)  # (b0+p)*T
    iotk = nc.declare_dram_parameter("iotk", [CHUNK, KSHORT], F32, isOutput=False)
    out = nc.declare_dram_parameter("out", [BL, D], F32, isOutput=True)
    hist_flat = hist.rearrange("b t d -> (b t) d")

    with tile.TileContext(nc) as tc:
        with (
            tc.tile_pool(name="hist", bufs=2) as hist_pool,
            tc.tile_pool(name="big", bufs=1) as big_pool,
            tc.tile_pool(name="scr2", bufs=2) as s2_pool,
            tc.tile_pool(name="small", bufs=1) as sm_pool,
            tc.tile_pool(name="dmain", bufs=2) as dm_pool,
            tc.tile_pool(name="tiny", bufs=3) as tn_pool,
            tc.tile_pool(name="const", bufs=1) as c_pool,
            tc.tile_pool(name="psum", bufs=4, space="PSUM") as ps_pool,
        ):
            # constants
            w2_sb = c_pool.tile([D, D], F32)
            nc.sync.dma_start(out=w2_sb, in_=w2[:, :])
            wvt_sb = c_pool.tile([D, D], F32)
            nc.sync.dma_start(out=wvt_sb, in_=wvt[:, :])
            gam_sb = c_pool.tile([CHUNK, D], F32)
            nc.sync.dma_start(out=gam_sb, in_=gam[:, :])
            bet_sb = c_pool.tile([CHUNK, D], F32)
            nc.sync.dma_start(out=bet_sb, in_=bet[:, :])
            iot_sb = c_pool.tile([CHUNK, T], F32)
            nc.sync.dma_start(out=iot_sb, in_=iot[:, :])
            rowb_sb = c_pool.tile([CHUNK, NCHUNK], F32)
            nc.sync.dma_start(out=rowb_sb, in_=rowb[:, :])
            iotk_sb = c_pool.tile([CHUNK, KSHORT], F32)
            nc.sync.dma_start(out=iotk_sb, in_=iotk[:, :])
            ident = c_pool.tile([CHUNK, CHUNK], F32)
            make_identity(nc, ident)
            zero_c = c_pool.tile([CHUNK, 1], F32)
            nc.vector.memset(zero_c, 0.0)
            tiny_c = c_pool.tile([CHUNK, 1], F32)
            nc.vector.memset(tiny_c, 1e-12)
            tc.strict_bb_all_engine_barrier()

            for c in range(NCHUNK):
                b0 = c * CHUNK
                # ---- loads (hist cast f32->bf16 via SWDGE) ----
                h_bf = hist_pool.tile([CHUNK, T, D], BF16)
                for s in range(NTSUB):
                    nc.gpsimd.dma_start(
                        out=h_bf[:, s * TSUB:(s + 1) * TSUB, :],
                        in_=hist[b0:b0 + CHUNK, s * TSUB:(s + 1) * TSUB, :],
                    )
                mask_u8 = dm_pool.tile([CHUNK, T], U8, tag="mask_u8")
                nc.sync.dma_start(out=mask_u8, in_=mask[b0:b0 + CHUNK, :])
                age_f = dm_pool.tile([CHUNK, T], F32, tag="age")
                nc.sync.dma_start(out=age_f, in_=age[b0:b0 + CHUNK, :])
                pop_f = dm_pool.tile([CHUNK, T], F32, tag="pop")
                nc.sync.dma_start(out=pop_f, in_=pop[b0:b0 + CHUNK, :])

                # ---- small prep ----
                maskf = sm_pool.tile([CHUNK, T], F32, tag="maskf")
                nc.vector.tensor_copy(maskf, mask_u8)
                mask2 = sm_pool.tile([CHUNK, T, 2], BF16, tag="mask2")
                nc.vector.tensor_copy(
                    mask2, bass.AP(tensor=mask_u8.tensor, offset=mask_u8.offset,
                                   ap=[mask_u8.ap[0], [1, T], [0, 2]]))
                msum = tn_pool.tile([CHUNK, 1], F32, tag="msum")
                nc.vector.reduce_sum(msum, maskf, axis=AX.X)
                mden = tn_pool.tile([CHUNK, 1], F32, tag="mden")
                nc.vector.tensor_scalar_add(mden, msum, 1e-6)
                mrec = tn_pool.tile([CHUNK, 1], F32, tag="mrec")
                nc.vector.reciprocal(mrec, mden)

                # decay log-bias: dl = log(exp(-alpha*age) + 1e-12)
                edec = sm_pool.tile([CHUNK, T], F32, tag="edec")
                nc.scalar.activation(edec, age_f, ACT.Exp, bias=zero_c, scale=-alpha)
                dl = sm_pool.tile([CHUNK, T], F32, tag="dl")
                nc.scalar.activation(dl, edec, ACT.Ln, bias=tiny_c)

                # ---- last-K gather: rows [s0, s0+5) with s0 = max(cnt-5, 0) ----
                cnt = tn_pool.tile([CHUNK, 1], F32, tag="cnt")
                nc.vector.tensor_scalar_max(cnt, msum, 1.0)
                s0 = tn_pool.tile([CHUNK, 1], F32, tag="s0")
                nc.vector.tensor_scalar(s0, cnt, -float(KSHORT), 0.0,
                                        op0=ALU.add, op1=ALU.max)
                gidx_f = tn_pool.tile([CHUNK, 1], F32, tag="gidx_f")
                nc.vector.tensor_add(gidx_f, s0, rowb_sb[:, c:c + 1])
                gidx = tn_pool.tile([CHUNK, 1], I32, tag="gidx")
                nc.vector.tensor_copy(gidx, gidx_f)
                gath = sm_pool.tile([CHUNK, KSHORT, D], F32, tag="gath")
                nc.gpsimd.indirect_dma_start(
                    out=gath.rearrange("p k d -> p (k d)"),
                    out_offset=None,
                    in_=hist_flat,
                    in_offset=bass.IndirectOffsetOnAxis(ap=gidx, axis=0),
                )

                # ---- P1: mean = fold_t(maskf*hist) / (msum+1e-6) ----
                macc = sm_pool.tile([CHUNK, D], F32, tag="macc")
                for h in range(2):
                    th = T // 2
                    scr = big_pool.tile([CHUNK, T // 2, D], BF16, tag="p1scr")
                    nc.vector.tensor_mul(
                        _pairs(scr, 0, th), _pairs(h_bf, h * th, th),
                        _bc_pair(mask2, h * th, th))
                    hpart = sm_pool.tile([CHUNK, D], F32, tag="hpart")
                    _fold_t(nc, scr, th, hpart, tn_pool)
                    if h == 0:
                        nc.vector.tensor_copy(macc, hpart)
                    else:
                        nc.vector.tensor_add(macc, macc, hpart)
                mean_sb = sm_pool.tile([CHUNK, D], F32, tag="mean")
                nc.vector.tensor_scalar_mul(mean_sb, macc, mrec)

                # ---- qk = mean @ W2 (PE) ----
                meanT_ps = ps_pool.tile([D, CHUNK], F32, tag="tp")
                nc.tensor.transpose(meanT_ps, mean_sb, ident)
                meanT_sb = sm_pool.tile([D, CHUNK], F32, tag="meanT")
                nc.scalar.copy(meanT_sb, meanT_ps)
                qk_ps = ps_pool.tile([CHUNK, D], F32, tag="mm")
                nc.tensor.matmul(qk_ps, meanT_sb, w2_sb, start=True, stop=True)
                qk_bf = sm_pool.tile([CHUNK, D], BF16, tag="qk")
                nc.scalar.copy(qk_bf, qk_ps)

                # ---- P2: scores = fold_d(qk*hist) + dl, mask -> softmax ----
                scores = sm_pool.tile([CHUNK, T], F32, tag="scores")
                for s in range(NTSUB):
                    t0 = s * TSUB
                    stmp = s2_pool.tile([CHUNK, TSUB, D], BF16, tag="scr2")
                    nc.vector.tensor_mul(
                        stmp, h_bf[:, t0:t0 + TSUB, :], _bc(qk_bf, TSUB, "mid"))
                    _fold_d(nc, stmp, TSUB, scores[:, t0:t0 + TSUB])
                nc.vector.tensor_add(scores, scores, dl)
                smask = sm_pool.tile([CHUNK, T], F32, tag="smask")
                nc.vector.memset(smask, -1e30)
                nc.vector.copy_predicated(smask, mask_u8, scores)
                smax = tn_pool.tile([CHUNK, 1], F32, tag="smax")
                nc.vector.tensor_reduce(smax, smask, axis=AX.X, op=ALU.max,
                                        negate=True)
                esc_bf = sm_pool.tile([CHUNK, T], BF16, tag="esc_bf")
                ssum = tn_pool.tile([CHUNK, 1], F32, tag="ssum")
                nc.scalar.activation(esc_bf, smask, ACT.Exp, bias=smax,
                                     accum_out=ssum)
                esc2 = sm_pool.tile([CHUNK, T, 2], BF16, tag="esc2")
                nc.vector.tensor_copy(
                    esc2, bass.AP(tensor=esc_bf.tensor, offset=esc_bf.offset,
                                  ap=[esc_bf.ap[0], [1, T], [0, 2]]))
                srec = tn_pool.tile([CHUNK, 1], F32, tag="srec")
                nc.vector.reciprocal(srec, ssum)

                # ---- P3: wh = fold_t(esc*hist) * srec ; long = wh @ Wv.T ----
                wacc = sm_pool.tile([CHUNK, D], F32, tag="wacc")
                for h in range(4):
                    th = T // 4
                    scr3 = s2_pool.tile([CHUNK, T // 4, D], BF16, tag="p3scr")
                    nc.vector.tensor_mul(
                        _pairs(scr3, 0, th), _pairs(h_bf, h * th, th),
                        _bc_pair(esc2, h * th, th))
                    hpart = sm_pool.tile([CHUNK, D], F32, tag="whpart")
                    _fold_t(nc, scr3, th, hpart, tn_pool)
                    if h == 0:
                        nc.vector.tensor_copy(wacc, hpart)
                    else:
                        nc.vector.tensor_add(wacc, wacc, hpart)
                wh_sb = sm_pool.tile([CHUNK, D], F32, tag="wh")
                nc.vector.tensor_scalar_mul(wh_sb, wacc, srec)
                whT_ps = ps_pool.tile([D, CHUNK], F32, tag="tp")
                nc.tensor.transpose(whT_ps, wh_sb, ident)
                whT_sb = sm_pool.tile([D, CHUNK], F32, tag="whT")
                nc.scalar.copy(whT_sb, whT_ps)
                long_ps = ps_pool.tile([CHUNK, D], F32, tag="mm")
                nc.tensor.matmul(long_ps, whT_sb, wvt_sb, start=True, stop=True)
                long_sb = sm_pool.tile([CHUNK, D], F32, tag="long")
                nc.scalar.copy(long_sb, long_ps)

                # ---- short term from gathered rows ----
                denom = tn_pool.tile([CHUNK, 1], F32, tag="denom")
                nc.vector.tensor_scalar_min(denom, cnt, float(KSHORT))
                drec = tn_pool.tile([CHUNK, 1], F32, tag="drec")
                nc.vector.reciprocal(drec, denom)
                sacc = sm_pool.tile([CHUNK, D], F32, tag="sacc")
                cs = tn_pool.tile([CHUNK, 1], F32, tag="cs")
                nc.vector.tensor_tensor(cs, cnt, s0, op=ALU.subtract)
                wj5 = tn_pool.tile([CHUNK, KSHORT], F32, tag="wj5")
                nc.vector.tensor_scalar(wj5, iotk_sb, cs, None, op0=ALU.is_lt)
                gj = sm_pool.tile([CHUNK, D], F32, tag="gj")
                for j in range(KSHORT):
                    nc.vector.tensor_scalar_mul(gj, gath[:, j, :],
                                                wj5[:, j:j + 1])
                    if j == 0:
                        nc.vector.tensor_copy(sacc, gj)
                    else:
                        nc.vector.tensor_add(sacc, sacc, gj)
                short_sb = sm_pool.tile([CHUNK, D], F32, tag="short")
                nc.vector.tensor_scalar_mul(short_sb, sacc, drec)

                # ---- window means of pop/age -> gate ----
                lkg = sm_pool.tile([CHUNK, T], F32, tag="lkg")
                nc.vector.tensor_scalar(lkg, iot_sb, s0, None, op0=ALU.is_ge)
                lkl = sm_pool.tile([CHUNK, T], F32, tag="lkl")
                nc.vector.tensor_scalar(lkl, iot_sb, cnt, None, op0=ALU.is_lt)
                lk = sm_pool.tile([CHUNK, T], F32, tag="lk")
                nc.vector.tensor_mul(lk, lkg, lkl)
                lp = sm_pool.tile([CHUNK, T], F32, tag="lp")
                nc.vector.tensor_mul(lp, lk, pop_f)
                mp = tn_pool.tile([CHUNK, 1], F32, tag="mp")
                nc.vector.reduce_sum(mp, lp, axis=AX.X)
                nc.vector.tensor_mul(lp, lk, age_f)
                mr = tn_pool.tile([CHUNK, 1], F32, tag="mr")
                nc.vector.reduce_sum(mr, lp, axis=AX.X)
                z1 = tn_pool.tile([CHUNK, 1], F32, tag="z1")
                nc.vector.tensor_scalar_mul(z1, mp, gw0)
                z2 = tn_pool.tile([CHUNK, 1], F32, tag="z2")
                nc.vector.tensor_scalar_mul(z2, mr, gw1)
                nc.vector.tensor_add(z1, z1, z2)
                nc.vector.tensor_scalar_mul(z1, z1, drec)
                nc.vector.tensor_scalar_add(z1, z1, gb)
                ez = tn_pool.tile([CHUNK, 1], F32, tag="ez")
                nc.scalar.activation(ez, z1, ACT.Exp, bias=zero_c, scale=-1.0)
                ez1 = tn_pool.tile([CHUNK, 1], F32, tag="ez1")
                nc.vector.tensor_scalar_add(ez1, ez, 1.0)
                g = tn_pool.tile([CHUNK, 1], F32, tag="g")
                nc.vector.reciprocal(g, ez1)
                omg = tn_pool.tile([CHUNK, 1], F32, tag="omg")
                nc.vector.tensor_mul(omg, ez, g)

                # ---- combine + layernorm ----
                user = sm_pool.tile([CHUNK, D], F32, tag="user")
                nc.vector.tensor_scalar_mul(user, short_sb, g)
                ulong = sm_pool.tile([CHUNK, D], F32, tag="ulong")
                nc.vector.tensor_scalar_mul(ulong, long_sb, omg)
                nc.vector.tensor_add(user, user, ulong)

                stats = tn_pool.tile([CHUNK, 6], F32, tag="stats")
                nc.vector.bn_stats(stats, user)
                mv = tn_pool.tile([CHUNK, 2], F32, tag="mv")
                nc.vector.bn_aggr(mv, stats)
                veps = tn_pool.tile([CHUNK, 1], F32, tag="veps")
                nc.vector.tensor_scalar_add(veps, mv[:, 1:2], 1e-5)
                vrec = tn_pool.tile([CHUNK, 1], F32, tag="vrec")
                nc.vector.reciprocal(vrec, veps)  # 1/(var+eps)
                lnv = tn_pool.tile([CHUNK, 1], F32, tag="lnv")
                nc.scalar.activation(lnv, vrec, ACT.Ln, bias=tiny_c)
                rstd = tn_pool.tile([CHUNK, 1], F32, tag="rstd")
                nc.scalar.activation(rstd, lnv, ACT.Exp, bias=zero_c, scale=0.5)
                negmur = tn_pool.tile([CHUNK, 1], F32, tag="negmur")
                nc.vector.tensor_scalar(negmur, mv[:, 0:1], -1.0, rstd,
                                        op0=ALU.mult, op1=ALU.mult)
                usern = sm_pool.tile([CHUNK, D], F32, tag="usern")
                nc.scalar.activation(usern, user, ACT.Identity, bias=negmur,
                                     scale=rstd)
                nc.vector.tensor_mul(usern, usern, gam_sb)
                ou = sm_pool.tile([CHUNK, D], F32, tag="ou")
                nc.vector.tensor_add(ou, usern, bet_sb)
                nc.sync.dma_start(out=out[b0:b0 + CHUNK, :], in_=ou)

    nc.finalize()
    return nc


def _get_nc(alpha, gw0, gw1, gb):
    key = (round(alpha, 10), round(gw0, 10), round(gw1, 10), round(gb, 10))
    if key not in _CACHE:
        _CACHE[key] = _build(alpha, gw0, gw1, gb)
    return _CACHE[key]


def _run(inputs, trace=False):
    hist = np.ascontiguousarray(inputs["hist_items"], dtype=np.float32)
    mask = np.ascontiguousarray(inputs["hist_mask"]).astype(np.uint8)
    age = np.ascontiguousarray(inputs["hist_age_hours"], dtype=np.float32)
    pop = np.ascontiguousarray(inputs["hist_popularity"], dtype=np.float32)
    Wq = np.asarray(inputs["Wq"], dtype=np.float32)
    Wk = np.asarray(inputs["Wk"], dtype=np.float32)
    Wv = np.asarray(inputs["Wv"], dtype=np.float32)
    gate_w = np.asarray(inputs["gate_w"], dtype=np.float32)
    gate_b = np.asarray(inputs["gate_b"], dtype=np.float32)
    ln_gamma = np.asarray(inputs["ln_gamma"], dtype=np.float32)
    ln_beta = np.asarray(inputs["ln_beta"], dtype=np.float32)
    decay_alpha = float(np.asarray(inputs["decay_alpha"]))

    alpha = _softplus(decay_alpha) + 1e-6
    gw0, gw1 = float(gate_w[0, 0]), float(gate_w[0, 1])
    gb = float(gate_b[0])
    w2 = (Wq.T @ Wk) / np.sqrt(D)
    wvt = np.ascontiguousarray(Wv.T)
    gam = np.broadcast_to(ln_gamma, (CHUNK, D)).copy()
    bet = np.broadcast_to(ln_beta, (CHUNK, D)).copy()
    iot = np.broadcast_to(np.arange(T, dtype=np.float32), (CHUNK, T)).copy()
    rowb = np.empty((CHUNK, NCHUNK), np.float32)
    for c in range(NCHUNK):
        rowb[:, c] = (c * CHUNK + np.arange(CHUNK)) * T
    iotk = np.broadcast_to(np.arange(KSHORT, dtype=np.float32),
                           (CHUNK, KSHORT)).copy()

    nc = _get_nc(alpha, gw0, gw1, gb)
    in_maps = []
    for i in range(NCORES):
        sl = slice(i * BL, (i + 1) * BL)
        in_maps.append({
            "hist": hist[sl], "mask": mask[sl], "age": age[sl], "pop": pop[sl],
            "w2": w2, "wvt": wvt, "gam": gam, "bet": bet, "iot": iot,
            "rowb": rowb, "iotk": iotk,
        })
    res = run_bass_kernel_spmd(nc, in_maps, core_ids=list(range(NCORES)),
                               trace=trace)
    outs = [res.results[i]["out"] for i in range(NCORES)]
    full = np.concatenate(outs, axis=0).astype(np.float32)
    return full, res


def kernel(**inputs):
    return _run(inputs)[0]


# revision 14
# speedup vs baseline: 1.5207x; 1.1504x over previous
"""ARIG user-encoder Trainium2 kernel (8-core pure data parallel).

B=4096, T=200, D=128. Each core handles 512 batches, processed as 4 chunks
of 128 (partition = batch). Weighted reductions over hist are DVE
broadcast-multiplies (bf16) + contiguous pairwise tree-folds (bf16 2x mode).
The last-K window is fetched with one indirect-DMA gather of the 5
contiguous rows ending at cnt. Tiny matmuls run on PE with host-prefolded
weights:
  qk = mean_hist @ (Wq.T @ Wk) * 1/sqrt(D)   (scores = hist . qk + log decay)
  long_term = wh @ Wv.T,  wh = sum_t attn*hist
"""

import sys

sys.path.insert(0, "/opt/trn_rl_repo")

import numpy as np

import concourse.bass as bass
import concourse.bacc as bacc
import concourse.tile as tile
from concourse import mybir
from concourse.bass_utils import run_bass_kernel_spmd
from concourse.masks import make_identity

B, T, D = 4096, 200, 128
KSHORT = 5
NCORES = 8
BL = B // NCORES          # 512 batches per core
CHUNK = 128               # batches per chunk (partition dim)
NCHUNK = BL // CHUNK      # 4
TSUB = 50                 # t subtile
NTSUB = T // TSUB         # 4

F32 = mybir.dt.float32
BF16 = mybir.dt.bfloat16
U8 = mybir.dt.uint8
I32 = mybir.dt.int32
AX = mybir.AxisListType
ALU = mybir.AluOpType
ACT = mybir.ActivationFunctionType

_CACHE = {}


def _softplus(x):
    return np.log1p(np.exp(-abs(x))) + max(x, 0.0)


def _bc(ap, n, where):
    """Insert a 0-stride broadcast dim of size n into a 2D [p, f] AP.
    where='mid' -> [p, n, f]; where='inner' -> [p, f, n]."""
    if where == "mid":
        dims = [ap.ap[0], [0, n], ap.ap[1]]
    else:
        dims = [ap.ap[0], ap.ap[1], [0, n]]
    return bass.AP(tensor=ap.tensor, offset=ap.offset, ap=dims)



def _bc_pair(ap2, toff, tlen):
    """AP over a duplicated-weights tile w2[p, T, 2] (w duplicated along last
    axis) shaped [p, tlen, D//2, 2] with stride-0 on the D//2 dim and step-1
    innermost pair -> eligible for DVE 2x packing."""
    p = ap2.ap[0]
    return bass.AP(tensor=ap2.tensor, offset=ap2.offset + toff * 2,
                   ap=[p, [2, tlen], [0, D // 2], [1, 2]])


def _pairs(ap3, toff, tlen):
    """View h/scr tile AP [p, T?, D] as [p, tlen, D//2, 2] starting at toff."""
    p = ap3.ap[0]
    return bass.AP(tensor=ap3.tensor, offset=ap3.offset + toff * D,
                   ap=[p, [D, tlen], [2, D // 2], [1, 2]])

def _fold_t(nc, scr, tlen, out, tmp_pool):
    """Sum scr[:, 0:tlen, :] over axis t by contiguous pairwise folds.
    scr is [128, T?, D] bf16 (destroyed). Result added... written to out
    ([128, D] f32) by the final fold."""
    # fold down by halves (in place), odd handled by folding the tail
    cur = tlen
    while cur > 2:
        half = cur // 2
        rem = cur - 2 * half  # 0 or 1
        # scr[:, 0:half] += scr[:, half:2*half]
        nc.vector.tensor_add(scr[:, 0:half, :], scr[:, 0:half, :],
                             scr[:, half:2 * half, :])
        if rem:
            # fold the leftover slice into position 0
            nc.vector.tensor_add(scr[:, 0:1, :], scr[:, 0:1, :],
                                 scr[:, cur - 1:cur, :])
        cur = half
    if cur == 2:
        nc.vector.tensor_add(out, scr[:, 0, :], scr[:, 1, :])
    else:
        nc.vector.tensor_copy(out, scr[:, 0, :])



def _fold_t_gp(nc, scr, tlen, out):
    """_fold_t on the gpsimd engine."""
    cur = tlen
    while cur > 2:
        half = cur // 2
        rem = cur - 2 * half
        nc.gpsimd.tensor_tensor(scr[:, 0:half, :], scr[:, 0:half, :],
                                scr[:, half:2 * half, :], op=ALU.add)
        if rem:
            nc.gpsimd.tensor_tensor(scr[:, 0:1, :], scr[:, 0:1, :],
                                    scr[:, cur - 1:cur, :], op=ALU.add)
        cur = half
    if cur == 2:
        nc.gpsimd.tensor_tensor(out, scr[:, 0, :], scr[:, 1, :], op=ALU.add)
    else:
        nc.gpsimd.tensor_copy(out, scr[:, 0, :])

def _fold_d(nc, scr, tlen, out):
    """Sum scr[:, 0:tlen, 0:128] over inner d by contiguous pairwise folds;
    writes out [128, tlen] f32."""
    cur = D
    while cur > 2:
        half = cur // 2
        nc.vector.tensor_add(scr[:, 0:tlen, 0:half], scr[:, 0:tlen, 0:half],
                             scr[:, 0:tlen, half:2 * half])
        cur = half
    nc.vector.tensor_add(out, scr[:, 0:tlen, 0], scr[:, 0:tlen, 1])


def _build(alpha, gw0, gw1, gb):
    nc = bacc.Bacc("TRN2")

    hist = nc.declare_dram_parameter("hist", [BL, T, D], F32, isOutput=False)
    mask = nc.declare_dram_parameter("mask", [BL, T], U8, isOutput=False)
    age = nc.declare_dram_parameter("age", [BL, T], F32, isOutput=False)
    pop = nc.declare_dram_parameter("pop", [BL, T], F32, isOutput=False)
    w2 = nc.declare_dram_parameter("w2", [D, D], F32, isOutput=False)      # Wq.T@Wk/sqrt(D)
    wvt = nc.declare_dram_parameter("wvt", [D, D], F32, isOutput=False)    # Wv.T
    gam = nc.declare_dram_parameter("gam", [CHUNK, D], F32, isOutput=False)  # gamma bcast
    bet = nc.declare_dram_parameter("bet", [CHUNK, D], F32, isOutput=False)  # beta bcast
    iot = nc.declare_dram_parameter("iot", [CHUNK, T], F32, isOutput=False)  # arange(T) bcast
    rowb = nc.declare_dram_parameter("rowb", [CHUNK, NCHUNK], F32, isOutput=False)  # (b0+p)*T
    iotk = nc.declare_dram_parameter("iotk", [CHUNK, KSHORT], F32, isOutput=False)
    out = nc.declare_dram_parameter("out", [BL, D], F32, isOutput=True)
    hist_flat = hist.rearrange("b t d -> (b t) d")

    with tile.TileContext(nc) as tc:
        with (
            tc.tile_pool(name="hist", bufs=2) as hist_pool,
            tc.tile_pool(name="big", bufs=1) as big_pool,
            tc.tile_pool(name="scr2", bufs=2) as s2_pool,
            tc.tile_pool(name="small", bufs=1) as sm_pool,
            tc.tile_pool(name="dmain", bufs=2) as dm_pool,
            tc.tile_pool(name="tiny", bufs=3) as tn_pool,
            tc.tile_pool(name="const", bufs=1) as c_pool,
            tc.tile_pool(name="psum", bufs=4, space="PSUM") as ps_pool,
        ):
            # constants
            w2_sb = c_pool.tile([D, D], F32)
            nc.sync.dma_start(out=w2_sb, in_=w2[:, :])
            wvt_sb = c_pool.tile([D, D], F32)
            nc.sync.dma_start(out=wvt_sb, in_=wvt[:, :])
            gam_sb = c_pool.tile([CHUNK, D], F32)
            nc.sync.dma_start(out=gam_sb, in_=gam[:, :])
            bet_sb = c_pool.tile([CHUNK, D], F32)
            nc.sync.dma_start(out=bet_sb, in_=bet[:, :])
            iot_sb = c_pool.tile([CHUNK, T], F32)
            nc.sync.dma_start(out=iot_sb, in_=iot[:, :])
            rowb_sb = c_pool.tile([CHUNK, NCHUNK], F32)
            nc.sync.dma_start(out=rowb_sb, in_=rowb[:, :])
            iotk_sb = c_pool.tile([CHUNK, KSHORT], F32)
            nc.sync.dma_start(out=iotk_sb, in_=iotk[:, :])
            ident = c_pool.tile([CHUNK, CHUNK], F32)
            make_identity(nc, ident)
            zero_c = c_pool.tile([CHUNK, 1], F32)
            nc.vector.memset(zero_c, 0.0)
            tiny_c = c_pool.tile([CHUNK, 1], F32)
            nc.vector.memset(tiny_c, 1e-12)
            tc.strict_bb_all_engine_barrier()

            for c in range(NCHUNK):
                b0 = c * CHUNK
                # ---- loads (hist cast f32->bf16 via SWDGE) ----
                h_sub = []
                for s in range(NTSUB):
                    hs = hist_pool.tile([CHUNK, TSUB, D], BF16, tag=f"hs{s}")
                    nc.gpsimd.dma_start(
                        out=hs,
                        in_=hist[b0:b0 + CHUNK, s * TSUB:(s + 1) * TSUB, :],
                    )
                    h_sub.append(hs)
                mask_u8 = dm_pool.tile([CHUNK, T], U8, tag="mask_u8")
                nc.sync.dma_start(out=mask_u8, in_=mask[b0:b0 + CHUNK, :])
                age_f = dm_pool.tile([CHUNK, T], F32, tag="age")
                nc.sync.dma_start(out=age_f, in_=age[b0:b0 + CHUNK, :])
                pop_f = dm_pool.tile([CHUNK, T], F32, tag="pop")
                nc.sync.dma_start(out=pop_f, in_=pop[b0:b0 + CHUNK, :])

                # ---- small prep ----
                maskf = sm_pool.tile([CHUNK, T], F32, tag="maskf")
                nc.vector.tensor_copy(maskf, mask_u8)
                mask2 = sm_pool.tile([CHUNK, T, 2], BF16, tag="mask2")
                nc.vector.tensor_copy(
                    mask2, bass.AP(tensor=mask_u8.tensor, offset=mask_u8.offset,
                                   ap=[mask_u8.ap[0], [1, T], [0, 2]]))
                msum = tn_pool.tile([CHUNK, 1], F32, tag="msum")
                nc.vector.reduce_sum(msum, maskf, axis=AX.X)
                mden = tn_pool.tile([CHUNK, 1], F32, tag="mden")
                nc.vector.tensor_scalar_add(mden, msum, 1e-6)
                mrec = tn_pool.tile([CHUNK, 1], F32, tag="mrec")
                nc.vector.reciprocal(mrec, mden)

                # decay log-bias: dl = log(exp(-alpha*age) + 1e-12)
                edec = sm_pool.tile([CHUNK, T], F32, tag="edec")
                nc.scalar.activation(edec, age_f, ACT.Exp, bias=zero_c, scale=-alpha)
                dl = sm_pool.tile([CHUNK, T], F32, tag="dl")
                nc.scalar.activation(dl, edec, ACT.Ln, bias=tiny_c)

                # ---- last-K gather: rows [s0, s0+5) with s0 = max(cnt-5, 0) ----
                cnt = tn_pool.tile([CHUNK, 1], F32, tag="cnt")
                nc.vector.tensor_scalar_max(cnt, msum, 1.0)
                s0 = tn_pool.tile([CHUNK, 1], F32, tag="s0")
                nc.vector.tensor_scalar(s0, cnt, -float(KSHORT), 0.0,
                                        op0=ALU.add, op1=ALU.max)
                gidx_f = tn_pool.tile([CHUNK, 1], F32, tag="gidx_f")
                nc.vector.tensor_add(gidx_f, s0, rowb_sb[:, c:c + 1])
                gidx = tn_pool.tile([CHUNK, 1], I32, tag="gidx")
                nc.vector.tensor_copy(gidx, gidx_f)
                gath = sm_pool.tile([CHUNK, KSHORT, D], F32, tag="gath")
                nc.gpsimd.indirect_dma_start(
                    out=gath.rearrange("p k d -> p (k d)"),
                    out_offset=None,
                    in_=hist_flat,
                    in_offset=bass.IndirectOffsetOnAxis(ap=gidx, axis=0),
                )

                # ---- P1: mean = fold_t(maskf*hist) / (msum+1e-6) ----
                macc = sm_pool.tile([CHUNK, D], F32, tag="macc")
                for h in range(NTSUB):
                    th = TSUB
                    scr = big_pool.tile([CHUNK, TSUB, D], BF16, tag="p1scr")
                    nc.vector.tensor_mul(
                        _pairs(scr, 0, th), _pairs(h_sub[h], 0, th),
                        _bc_pair(mask2, h * th, th))
                    hpart = sm_pool.tile([CHUNK, D], F32, tag="hpart")
                    _fold_t(nc, scr, th, hpart, tn_pool)
                    if h == 0:
                        nc.vector.tensor_copy(macc, hpart)
                    else:
                        nc.vector.tensor_add(macc, macc, hpart)
                mean_sb = sm_pool.tile([CHUNK, D], F32, tag="mean")
                nc.vector.tensor_scalar_mul(mean_sb, macc, mrec)

                # ---- qk = mean @ W2 (PE) ----
                meanT_ps = ps_pool.tile([D, CHUNK], F32, tag="tp")
                nc.tensor.transpose(meanT_ps, mean_sb, ident)
                meanT_sb = sm_pool.tile([D, CHUNK], F32, tag="meanT")
                nc.scalar.copy(meanT_sb, meanT_ps)
                qk_ps = ps_pool.tile([CHUNK, D], F32, tag="mm")
                nc.tensor.matmul(qk_ps, meanT_sb, w2_sb, start=True, stop=True)
                qk_bf = sm_pool.tile([CHUNK, D], BF16, tag="qk")
                nc.scalar.copy(qk_bf, qk_ps)

                # ---- P2: scores = fold_d(qk*hist) + dl, mask -> softmax ----
                scores = sm_pool.tile([CHUNK, T], F32, tag="scores")
                for s in range(NTSUB):
                    t0 = s * TSUB
                    stmp = s2_pool.tile([CHUNK, TSUB, D], BF16, tag="scr2")
                    nc.vector.tensor_mul(stmp, h_sub[s], _bc(qk_bf, TSUB, "mid"))
                    _fold_d(nc, stmp, TSUB, scores[:, t0:t0 + TSUB])
                nc.vector.tensor_add(scores, scores, dl)
                smask = sm_pool.tile([CHUNK, T], F32, tag="smask")
                nc.vector.memset(smask, -1e30)
                nc.vector.copy_predicated(smask, mask_u8, scores)
                smax = tn_pool.tile([CHUNK, 1], F32, tag="smax")
                nc.vector.tensor_reduce(smax, smask, axis=AX.X, op=ALU.max,
                                        negate=True)
                esc_bf = sm_pool.tile([CHUNK, T], BF16, tag="esc_bf")
                ssum = tn_pool.tile([CHUNK, 1], F32, tag="ssum")
                nc.scalar.activation(esc_bf, smask, ACT.Exp, bias=smax,
                                     accum_out=ssum)
                esc2 = sm_pool.tile([CHUNK, T, 2], BF16, tag="esc2")
                nc.vector.tensor_copy(
                    esc2, bass.AP(tensor=esc_bf.tensor, offset=esc_bf.offset,
                                  ap=[esc_bf.ap[0], [1, T], [0, 2]]))
                srec = tn_pool.tile([CHUNK, 1], F32, tag="srec")
                nc.vector.reciprocal(srec, ssum)

                # ---- P3: wh = fold_t(esc*hist) * srec ; long = wh @ Wv.T ----
                wacc = sm_pool.tile([CHUNK, D], F32, tag="wacc")
                for h in range(4):
                    th = T // 4
                    scr3 = s2_pool.tile([CHUNK, T // 4, D], BF16, tag="p3scr")
                    nc.vector.tensor_mul(
                        _pairs(scr3, 0, th), _pairs(h_sub[h], 0, th),
                        _bc_pair(esc2, h * th, th))
                    hpart = sm_pool.tile([CHUNK, D], F32, tag="whpart")
                    _fold_t(nc, scr3, th, hpart, tn_pool)
                    if h == 0:
                        nc.vector.tensor_copy(wacc, hpart)
                    else:
                        nc.vector.tensor_add(wacc, wacc, hpart)
                wh_sb = sm_pool.tile([CHUNK, D], F32, tag="wh")
                nc.vector.tensor_scalar_mul(wh_sb, wacc, srec)
                whT_ps = ps_pool.tile([D, CHUNK], F32, tag="tp")
                nc.tensor.transpose(whT_ps, wh_sb, ident)
                whT_sb = sm_pool.tile([D, CHUNK], F32, tag="whT")
                nc.scalar.copy(whT_sb, whT_ps)
                long_ps = ps_pool.tile([CHUNK, D], F32, tag="mm")
                nc.tensor.matmul(long_ps, whT_sb, wvt_sb, start=True, stop=True)
                long_sb = sm_pool.tile([CHUNK, D], F32, tag="long")
                nc.scalar.copy(long_sb, long_ps)

                # ---- short term from gathered rows ----
                denom = tn_pool.tile([CHUNK, 1], F32, tag="denom")
                nc.vector.tensor_scalar_min(denom, cnt, float(KSHORT))
                drec = tn_pool.tile([CHUNK, 1], F32, tag="drec")
                nc.vector.reciprocal(drec, denom)
                sacc = sm_pool.tile([CHUNK, D], F32, tag="sacc")
                cs = tn_pool.tile([CHUNK, 1], F32, tag="cs")
                nc.vector.tensor_tensor(cs, cnt, s0, op=ALU.subtract)
                wj5 = tn_pool.tile([CHUNK, KSHORT], F32, tag="wj5")
                nc.vector.tensor_scalar(wj5, iotk_sb, cs, None, op0=ALU.is_lt)
                gj = sm_pool.tile([CHUNK, D], F32, tag="gj")
                for j in range(KSHORT):
                    nc.vector.tensor_scalar_mul(gj, gath[:, j, :],
                                                wj5[:, j:j + 1])
                    if j == 0:
                        nc.vector.tensor_copy(sacc, gj)
                    else:
                        nc.vector.tensor_add(sacc, sacc, gj)
                short_sb = sm_pool.tile([CHUNK, D], F32, tag="short")
                nc.vector.tensor_scalar_mul(short_sb, sacc, drec)

                # ---- window means of pop/age -> gate ----
                lkg = sm_pool.tile([CHUNK, T], F32, tag="lkg")
                nc.vector.tensor_scalar(lkg, iot_sb, s0, None, op0=ALU.is_ge)
                lkl = sm_pool.tile([CHUNK, T], F32, tag="lkl")
                nc.vector.tensor_scalar(lkl, iot_sb, cnt, None, op0=ALU.is_lt)
                lk = sm_pool.tile([CHUNK, T], F32, tag="lk")
                nc.vector.tensor_mul(lk, lkg, lkl)
                lp = sm_pool.tile([CHUNK, T], F32, tag="lp")
                nc.vector.tensor_mul(lp, lk, pop_f)
                mp = tn_pool.tile([CHUNK, 1], F32, tag="mp")
                nc.vector.reduce_sum(mp, lp, axis=AX.X)
                nc.vector.tensor_mul(lp, lk, age_f)
                mr = tn_pool.tile([CHUNK, 1], F32, tag="mr")
                nc.vector.reduce_sum(mr, lp, axis=AX.X)
                z1 = tn_pool.tile([CHUNK, 1], F32, tag="z1")
                nc.vector.tensor_scalar_mul(z1, mp, gw0)
                z2 = tn_pool.tile([CHUNK, 1], F32, tag="z2")
                nc.vector.tensor_scalar_mul(z2, mr, gw1)
                nc.vector.tensor_add(z1, z1, z2)
                nc.vector.tensor_scalar_mul(z1, z1, drec)
                nc.vector.tensor_scalar_add(z1, z1, gb)
                ez = tn_pool.tile([CHUNK, 1], F32, tag="ez")
                nc.scalar.activation(ez, z1, ACT.Exp, bias=zero_c, scale=-1.0)
                ez1 = tn_pool.tile([CHUNK, 1], F32, tag="ez1")
                nc.vector.tensor_scalar_add(ez1, ez, 1.0)
                g = tn_pool.tile([CHUNK, 1], F32, tag="g")
                nc.vector.reciprocal(g, ez1)
                omg = tn_pool.tile([CHUNK, 1], F32, tag="omg")
                nc.vector.tensor_mul(omg, ez, g)

                # ---- combine + layernorm ----
                user = sm_pool.tile([CHUNK, D], F32, tag="user")
                nc.vector.tensor_scalar_mul(user, short_sb, g)
                ulong = sm_pool.tile([CHUNK, D], F32, tag="ulong")
                nc.vector.tensor_scalar_mul(ulong, long_sb, omg)
                nc.vector.tensor_add(user, user, ulong)

                stats = tn_pool.tile([CHUNK, 6], F32, tag="stats")
                nc.vector.bn_stats(stats, user)
                mv = tn_pool.tile([CHUNK, 2], F32, tag="mv")
                nc.vector.bn_aggr(mv, stats)
                veps = tn_pool.tile([CHUNK, 1], F32, tag="veps")
                nc.vector.tensor_scalar_add(veps, mv[:, 1:2], 1e-5)
                vrec = tn_pool.tile([CHUNK, 1], F32, tag="vrec")
                nc.vector.reciprocal(vrec, veps)  # 1/(var+eps)
                lnv = tn_pool.tile([CHUNK, 1], F32, tag="lnv")
                nc.scalar.activation(lnv, vrec, ACT.Ln, bias=tiny_c)
                rstd = tn_pool.tile([CHUNK, 1], F32, tag="rstd")
                nc.scalar.activation(rstd, lnv, ACT.Exp, bias=zero_c, scale=0.5)
                negmur = tn_pool.tile([CHUNK, 1], F32, tag="negmur")
                nc.vector.tensor_scalar(negmur, mv[:, 0:1], -1.0, rstd,
                                        op0=ALU.mult, op1=ALU.mult)
                usern = sm_pool.tile([CHUNK, D], F32, tag="usern")
                nc.scalar.activation(usern, user, ACT.Identity, bias=negmur,
                                     scale=rstd)
                nc.vector.tensor_mul(usern, usern, gam_sb)
                ou = sm_pool.tile([CHUNK, D], F32, tag="ou")
                nc.vector.tensor_add(ou, usern, bet_sb)
                nc.sync.dma_start(out=out[b0:b0 + CHUNK, :], in_=ou)

    nc.finalize()
    return nc


def _get_nc(alpha, gw0, gw1, gb):
    key = (round(alpha, 10), round(gw0, 10), round(gw1, 10), round(gb, 10))
    if key not in _CACHE:
        _CACHE[key] = _build(alpha, gw0, gw1, gb)
    return _CACHE[key]


def _run(inputs, trace=False):
    hist = np.ascontiguousarray(inputs["hist_items"], dtype=np.float32)
    mask = np.ascontiguousarray(inputs["hist_mask"]).astype(np.uint8)
    age = np.ascontiguousarray(inputs["hist_age_hours"], dtype=np.float32)
    pop = np.ascontiguousarray(inputs["hist_popularity"], dtype=np.float32)
    Wq = np.asarray(inputs["Wq"], dtype=np.float32)
    Wk = np.asarray(inputs["Wk"], dtype=np.float32)
    Wv = np.asarray(inputs["Wv"], dtype=np.float32)
    gate_w = np.asarray(inputs["gate_w"], dtype=np.float32)
    gate_b = np.asarray(inputs["gate_b"], dtype=np.float32)
    ln_gamma = np.asarray(inputs["ln_gamma"], dtype=np.float32)
    ln_beta = np.asarray(inputs["ln_beta"], dtype=np.float32)
    decay_alpha = float(np.asarray(inputs["decay_alpha"]))

    alpha = _softplus(decay_alpha) + 1e-6
    gw0, gw1 = float(gate_w[0, 0]), float(gate_w[0, 1])
    gb = float(gate_b[0])
    w2 = (Wq.T @ Wk) / np.sqrt(D)
    wvt = np.ascontiguousarray(Wv.T)
    gam = np.broadcast_to(ln_gamma, (CHUNK, D)).copy()
    bet = np.broadcast_to(ln_beta, (CHUNK, D)).copy()
    iot = np.broadcast_to(np.arange(T, dtype=np.float32), (CHUNK, T)).copy()
    rowb = np.empty((CHUNK, NCHUNK), np.float32)
    for c in range(NCHUNK):
        rowb[:, c] = (c * CHUNK + np.arange(CHUNK)) * T
    iotk = np.broadcast_to(np.arange(KSHORT, dtype=np.float32),
                           (CHUNK, KSHORT)).copy()

    nc = _get_nc(alpha, gw0, gw1, gb)
    in_maps = []
    for i in range(NCORES):
        sl = slice(i * BL, (i + 1) * BL)
        in_maps.append({
            "hist": hist[sl], "mask": mask[sl], "age": age[sl], "pop": pop[sl],
            "w2": w2, "wvt": wvt, "gam": gam, "bet": bet, "iot": iot,
            "rowb": rowb, "iotk": iotk,
        })
    res = run_bass_kernel_spmd(nc, in_maps, core_ids=list(range(NCORES)),
                               trace=trace)
    outs = [res.results[i]["out"] for i in range(NCORES)]
    full = np.concatenate(outs, axis=0).astype(np.float32)
    return full, res


def kernel(**inputs):
    return _run(inputs)[0]


# revision 15
# speedup vs baseline: 1.5889x; 1.0449x over previous
"""ARIG user-encoder Trainium2 kernel (8-core pure data parallel).

B=4096, T=200, D=128. Each core handles 512 batches, processed as 4 chunks
of 128 (partition = batch). Weighted reductions over hist are DVE
broadcast-multiplies (bf16) + contiguous pairwise tree-folds (bf16 2x mode).
The last-K window is fetched with one indirect-DMA gather of the 5
contiguous rows ending at cnt. Tiny matmuls run on PE with host-prefolded
weights:
  qk = mean_hist @ (Wq.T @ Wk) * 1/sqrt(D)   (scores = hist . qk + log decay)
  long_term = wh @ Wv.T,  wh = sum_t attn*hist
"""

import sys

sys.path.insert(0, "/opt/trn_rl_repo")

import numpy as np

import concourse.bass as bass
import concourse.bacc as bacc
import concourse.tile as tile
from concourse import mybir
from concourse.bass_utils import run_bass_kernel_spmd
from concourse.masks import make_identity

B, T, D = 4096, 200, 128
KSHORT = 5
NCORES = 8
BL = B // NCORES          # 512 batches per core
CHUNK = 128               # batches per chunk (partition dim)
NCHUNK = BL // CHUNK      # 4
TSUB = 50                 # t subtile
NTSUB = T // TSUB         # 4

F32 = mybir.dt.float32
BF16 = mybir.dt.bfloat16
U8 = mybir.dt.uint8
I32 = mybir.dt.int32
AX = mybir.AxisListType
ALU = mybir.AluOpType
ACT = mybir.ActivationFunctionType

_CACHE = {}


def _softplus(x):
    return np.log1p(np.exp(-abs(x))) + max(x, 0.0)


def _bc(ap, n, where):
    """Insert a 0-stride broadcast dim of size n into a 2D [p, f] AP.
    where='mid' -> [p, n, f]; where='inner' -> [p, f, n]."""
    if where == "mid":
        dims = [ap.ap[0], [0, n], ap.ap[1]]
    else:
        dims = [ap.ap[0], ap.ap[1], [0, n]]
    return bass.AP(tensor=ap.tensor, offset=ap.offset, ap=dims)



def _bc_pair(ap2, toff, tlen):
    """AP over a duplicated-weights tile w2[p, T, 2] (w duplicated along last
    axis) shaped [p, tlen, D//2, 2] with stride-0 on the D//2 dim and step-1
    innermost pair -> eligible for DVE 2x packing."""
    p = ap2.ap[0]
    return bass.AP(tensor=ap2.tensor, offset=ap2.offset + toff * 2,
                   ap=[p, [2, tlen], [0, D // 2], [1, 2]])


def _pairs(ap3, toff, tlen):
    """View h/scr tile AP [p, T?, D] as [p, tlen, D//2, 2] starting at toff."""
    p = ap3.ap[0]
    return bass.AP(tensor=ap3.tensor, offset=ap3.offset + toff * D,
                   ap=[p, [D, tlen], [2, D // 2], [1, 2]])


def _fold_to12(nc, scr):
    """In-place fold scr[:, 0:50, :] -> scr[:, 0:12, :] (50->25->12+tail)."""
    nc.vector.tensor_add(scr[:, 0:25, :], scr[:, 0:25, :], scr[:, 25:50, :])
    nc.vector.tensor_add(scr[:, 0:12, :], scr[:, 0:12, :], scr[:, 12:24, :])
    nc.vector.tensor_add(scr[:, 0:1, :], scr[:, 0:1, :], scr[:, 24:25, :])


def _fold12_final(nc, acc12, out):
    """acc12 [p, 12, D] bf16 -> out [p, D] f32."""
    nc.vector.tensor_add(acc12[:, 0:6, :], acc12[:, 0:6, :], acc12[:, 6:12, :])
    nc.vector.tensor_add(acc12[:, 0:3, :], acc12[:, 0:3, :], acc12[:, 3:6, :])
    nc.vector.tensor_add(acc12[:, 0:1, :], acc12[:, 0:1, :], acc12[:, 1:2, :])
    nc.vector.tensor_add(out, acc12[:, 0, :], acc12[:, 2, :])

def _fold_t(nc, scr, tlen, out, tmp_pool):
    """Sum scr[:, 0:tlen, :] over axis t by contiguous pairwise folds.
    scr is [128, T?, D] bf16 (destroyed). Result added... written to out
    ([128, D] f32) by the final fold."""
    # fold down by halves (in place), odd handled by folding the tail
    cur = tlen
    while cur > 2:
        half = cur // 2
        rem = cur - 2 * half  # 0 or 1
        # scr[:, 0:half] += scr[:, half:2*half]
        nc.vector.tensor_add(scr[:, 0:half, :], scr[:, 0:half, :],
                             scr[:, half:2 * half, :])
        if rem:
            # fold the leftover slice into position 0
            nc.vector.tensor_add(scr[:, 0:1, :], scr[:, 0:1, :],
                                 scr[:, cur - 1:cur, :])
        cur = half
    if cur == 2:
        nc.vector.tensor_add(out, scr[:, 0, :], scr[:, 1, :])
    else:
        nc.vector.tensor_copy(out, scr[:, 0, :])



def _fold_t_gp(nc, scr, tlen, out):
    """_fold_t on the gpsimd engine."""
    cur = tlen
    while cur > 2:
        half = cur // 2
        rem = cur - 2 * half
        nc.gpsimd.tensor_tensor(scr[:, 0:half, :], scr[:, 0:half, :],
                                scr[:, half:2 * half, :], op=ALU.add)
        if rem:
            nc.gpsimd.tensor_tensor(scr[:, 0:1, :], scr[:, 0:1, :],
                                    scr[:, cur - 1:cur, :], op=ALU.add)
        cur = half
    if cur == 2:
        nc.gpsimd.tensor_tensor(out, scr[:, 0, :], scr[:, 1, :], op=ALU.add)
    else:
        nc.gpsimd.tensor_copy(out, scr[:, 0, :])

def _fold_d(nc, scr, tlen, out):
    """Sum scr[:, 0:tlen, 0:128] over inner d by contiguous pairwise folds;
    writes out [128, tlen] f32."""
    cur = D
    while cur > 2:
        half = cur // 2
        nc.vector.tensor_add(scr[:, 0:tlen, 0:half], scr[:, 0:tlen, 0:half],
                             scr[:, 0:tlen, half:2 * half])
        cur = half
    nc.vector.tensor_add(out, scr[:, 0:tlen, 0], scr[:, 0:tlen, 1])


def _build(alpha, gw0, gw1, gb):
    nc = bacc.Bacc("TRN2")

    hist = nc.declare_dram_parameter("hist", [BL, T, D], F32, isOutput=False)
    mask = nc.declare_dram_parameter("mask", [BL, T], U8, isOutput=False)
    age = nc.declare_dram_parameter("age", [BL, T], F32, isOutput=False)
    pop = nc.declare_dram_parameter("pop", [BL, T], F32, isOutput=False)
    w2 = nc.declare_dram_parameter("w2", [D, D], F32, isOutput=False)      # Wq.T@Wk/sqrt(D)
    wvt = nc.declare_dram_parameter("wvt", [D, D], F32, isOutput=False)    # Wv.T
    gam = nc.declare_dram_parameter("gam", [CHUNK, D], F32, isOutput=False)  # gamma bcast
    bet = nc.declare_dram_parameter("bet", [CHUNK, D], F32, isOutput=False)  # beta bcast
    iot = nc.declare_dram_parameter("iot", [CHUNK, T], F32, isOutput=False)  # arange(T) bcast
    rowb = nc.declare_dram_parameter("rowb", [CHUNK, NCHUNK], F32, isOutput=False)  # (b0+p)*T
    iotk = nc.declare_dram_parameter("iotk", [CHUNK, KSHORT], F32, isOutput=False)
    out = nc.declare_dram_parameter("out", [BL, D], F32, isOutput=True)
    hist_flat = hist.rearrange("b t d -> (b t) d")

    with tile.TileContext(nc) as tc:
        with (
            tc.tile_pool(name="hist", bufs=2) as hist_pool,
            tc.tile_pool(name="big", bufs=1) as big_pool,
            tc.tile_pool(name="scr2", bufs=2) as s2_pool,
            tc.tile_pool(name="small", bufs=1) as sm_pool,
            tc.tile_pool(name="dmain", bufs=2) as dm_pool,
            tc.tile_pool(name="tiny", bufs=3) as tn_pool,
            tc.tile_pool(name="const", bufs=1) as c_pool,
            tc.tile_pool(name="psum", bufs=4, space="PSUM") as ps_pool,
        ):
            # constants
            w2_sb = c_pool.tile([D, D], F32)
            nc.sync.dma_start(out=w2_sb, in_=w2[:, :])
            wvt_sb = c_pool.tile([D, D], F32)
            nc.sync.dma_start(out=wvt_sb, in_=wvt[:, :])
            gam_sb = c_pool.tile([CHUNK, D], F32)
            nc.sync.dma_start(out=gam_sb, in_=gam[:, :])
            bet_sb = c_pool.tile([CHUNK, D], F32)
            nc.sync.dma_start(out=bet_sb, in_=bet[:, :])
            iot_sb = c_pool.tile([CHUNK, T], F32)
            nc.sync.dma_start(out=iot_sb, in_=iot[:, :])
            rowb_sb = c_pool.tile([CHUNK, NCHUNK], F32)
            nc.sync.dma_start(out=rowb_sb, in_=rowb[:, :])
            iotk_sb = c_pool.tile([CHUNK, KSHORT], F32)
            nc.sync.dma_start(out=iotk_sb, in_=iotk[:, :])
            ident = c_pool.tile([CHUNK, CHUNK], F32)
            make_identity(nc, ident)
            zero_c = c_pool.tile([CHUNK, 1], F32)
            nc.vector.memset(zero_c, 0.0)
            tiny_c = c_pool.tile([CHUNK, 1], F32)
            nc.vector.memset(tiny_c, 1e-12)
            tc.strict_bb_all_engine_barrier()

            for c in range(NCHUNK):
                b0 = c * CHUNK
                # ---- loads (hist cast f32->bf16 via SWDGE) ----
                mask_u8 = dm_pool.tile([CHUNK, T], U8, tag="mask_u8")
                nc.sync.dma_start(out=mask_u8, in_=mask[b0:b0 + CHUNK, :])
                age_f = dm_pool.tile([CHUNK, T], F32, tag="age")
                nc.sync.dma_start(out=age_f, in_=age[b0:b0 + CHUNK, :])
                pop_f = dm_pool.tile([CHUNK, T], F32, tag="pop")
                nc.sync.dma_start(out=pop_f, in_=pop[b0:b0 + CHUNK, :])
                h_sub = []
                for s in range(NTSUB):
                    hs = hist_pool.tile([CHUNK, TSUB, D], BF16, tag=f"hs{s}")
                    nc.gpsimd.dma_start(
                        out=hs,
                        in_=hist[b0:b0 + CHUNK, s * TSUB:(s + 1) * TSUB, :],
                    )
                    h_sub.append(hs)

                # ---- small prep ----
                maskf = sm_pool.tile([CHUNK, T], F32, tag="maskf")
                nc.vector.tensor_copy(maskf, mask_u8)
                mask2 = sm_pool.tile([CHUNK, T, 2], BF16, tag="mask2")
                nc.vector.tensor_copy(
                    mask2, bass.AP(tensor=mask_u8.tensor, offset=mask_u8.offset,
                                   ap=[mask_u8.ap[0], [1, T], [0, 2]]))
                msum = tn_pool.tile([CHUNK, 1], F32, tag="msum")
                nc.vector.reduce_sum(msum, maskf, axis=AX.X)
                mden = tn_pool.tile([CHUNK, 1], F32, tag="mden")
                nc.vector.tensor_scalar_add(mden, msum, 1e-6)
                mrec = tn_pool.tile([CHUNK, 1], F32, tag="mrec")
                nc.vector.reciprocal(mrec, mden)

                # decay log-bias: dl = log(exp(-alpha*age) + 1e-12)
                edec = sm_pool.tile([CHUNK, T], F32, tag="edec")
                nc.scalar.activation(edec, age_f, ACT.Exp, bias=zero_c, scale=-alpha)
                dl = sm_pool.tile([CHUNK, T], F32, tag="dl")
                nc.scalar.activation(dl, edec, ACT.Ln, bias=tiny_c)

                # ---- last-K gather: rows [s0, s0+5) with s0 = max(cnt-5, 0) ----
                cnt = tn_pool.tile([CHUNK, 1], F32, tag="cnt")
                nc.vector.tensor_scalar_max(cnt, msum, 1.0)
                s0 = tn_pool.tile([CHUNK, 1], F32, tag="s0")
                nc.vector.tensor_scalar(s0, cnt, -float(KSHORT), 0.0,
                                        op0=ALU.add, op1=ALU.max)
                gidx_f = tn_pool.tile([CHUNK, 1], F32, tag="gidx_f")
                nc.vector.tensor_add(gidx_f, s0, rowb_sb[:, c:c + 1])
                gidx = tn_pool.tile([CHUNK, 1], I32, tag="gidx")
                nc.vector.tensor_copy(gidx, gidx_f)
                gath = sm_pool.tile([CHUNK, KSHORT, D], F32, tag="gath")
                nc.gpsimd.indirect_dma_start(
                    out=gath.rearrange("p k d -> p (k d)"),
                    out_offset=None,
                    in_=hist_flat,
                    in_offset=bass.IndirectOffsetOnAxis(ap=gidx, axis=0),
                )

                # ---- P1: mean = fold_t(maskf*hist) / (msum+1e-6) ----
                macc = sm_pool.tile([CHUNK, D], F32, tag="macc")
                acc12 = sm_pool.tile([CHUNK, 12, D], BF16, tag="acc12")
                for h in range(NTSUB):
                    th = TSUB
                    scr = big_pool.tile([CHUNK, TSUB, D], BF16, tag="p1scr")
                    nc.vector.tensor_mul(
                        _pairs(scr, 0, th), _pairs(h_sub[h], 0, th),
                        _bc_pair(mask2, h * th, th))
                    _fold_to12(nc, scr)
                    if h == 0:
                        nc.vector.tensor_copy(acc12, scr[:, 0:12, :])
                    else:
                        nc.vector.tensor_add(acc12, acc12, scr[:, 0:12, :])
                _fold12_final(nc, acc12, macc)
                mean_sb = sm_pool.tile([CHUNK, D], F32, tag="mean")
                nc.vector.tensor_scalar_mul(mean_sb, macc, mrec)

                # ---- qk = mean @ W2 (PE) ----
                meanT_ps = ps_pool.tile([D, CHUNK], F32, tag="tp")
                nc.tensor.transpose(meanT_ps, mean_sb, ident)
                meanT_sb = sm_pool.tile([D, CHUNK], F32, tag="meanT")
                nc.scalar.copy(meanT_sb, meanT_ps)
                qk_ps = ps_pool.tile([CHUNK, D], F32, tag="mm")
                nc.tensor.matmul(qk_ps, meanT_sb, w2_sb, start=True, stop=True)
                qk_bf = sm_pool.tile([CHUNK, D], BF16, tag="qk")
                nc.scalar.copy(qk_bf, qk_ps)

                # ---- P2: scores = fold_d(qk*hist) + dl, mask -> softmax ----
                scores = sm_pool.tile([CHUNK, T], F32, tag="scores")
                for s in range(NTSUB):
                    t0 = s * TSUB
                    stmp = s2_pool.tile([CHUNK, TSUB, D], BF16, tag="scr2")
                    nc.vector.tensor_mul(stmp, h_sub[s], _bc(qk_bf, TSUB, "mid"))
                    _fold_d(nc, stmp, TSUB, scores[:, t0:t0 + TSUB])
                nc.vector.tensor_add(scores, scores, dl)
                smask = sm_pool.tile([CHUNK, T], F32, tag="smask")
                nc.vector.memset(smask, -1e30)
                nc.vector.copy_predicated(smask, mask_u8, scores)
                smax = tn_pool.tile([CHUNK, 1], F32, tag="smax")
                nc.vector.tensor_reduce(smax, smask, axis=AX.X, op=ALU.max,
                                        negate=True)
                esc_bf = sm_pool.tile([CHUNK, T], BF16, tag="esc_bf")
                ssum = tn_pool.tile([CHUNK, 1], F32, tag="ssum")
                nc.scalar.activation(esc_bf, smask, ACT.Exp, bias=smax,
                                     accum_out=ssum)
                esc2 = sm_pool.tile([CHUNK, T, 2], BF16, tag="esc2")
                nc.vector.tensor_copy(
                    esc2, bass.AP(tensor=esc_bf.tensor, offset=esc_bf.offset,
                                  ap=[esc_bf.ap[0], [1, T], [0, 2]]))
                srec = tn_pool.tile([CHUNK, 1], F32, tag="srec")
                nc.vector.reciprocal(srec, ssum)

                # ---- P3: wh = fold_t(esc*hist) * srec ; long = wh @ Wv.T ----
                wacc = sm_pool.tile([CHUNK, D], F32, tag="wacc")
                wcc12 = sm_pool.tile([CHUNK, 12, D], BF16, tag="wcc12")
                for h in range(4):
                    th = T // 4
                    scr3 = s2_pool.tile([CHUNK, T // 4, D], BF16, tag="p3scr")
                    nc.vector.tensor_mul(
                        _pairs(scr3, 0, th), _pairs(h_sub[h], 0, th),
                        _bc_pair(esc2, h * th, th))
                    _fold_to12(nc, scr3)
                    if h == 0:
                        nc.vector.tensor_copy(wcc12, scr3[:, 0:12, :])
                    else:
                        nc.vector.tensor_add(wcc12, wcc12, scr3[:, 0:12, :])
                _fold12_final(nc, wcc12, wacc)
                wh_sb = sm_pool.tile([CHUNK, D], F32, tag="wh")
                nc.vector.tensor_scalar_mul(wh_sb, wacc, srec)
                whT_ps = ps_pool.tile([D, CHUNK], F32, tag="tp")
                nc.tensor.transpose(whT_ps, wh_sb, ident)
                whT_sb = sm_pool.tile([D, CHUNK], F32, tag="whT")
                nc.scalar.copy(whT_sb, whT_ps)
                long_ps = ps_pool.tile([CHUNK, D], F32, tag="mm")
                nc.tensor.matmul(long_ps, whT_sb, wvt_sb, start=True, stop=True)
                long_sb = sm_pool.tile([CHUNK, D], F32, tag="long")
                nc.scalar.copy(long_sb, long_ps)

                # ---- short term from gathered rows ----
                denom = tn_pool.tile([CHUNK, 1], F32, tag="denom")
                nc.vector.tensor_scalar_min(denom, cnt, float(KSHORT))
                drec = tn_pool.tile([CHUNK, 1], F32, tag="drec")
                nc.vector.reciprocal(drec, denom)
                sacc = sm_pool.tile([CHUNK, D], F32, tag="sacc")
                cs = tn_pool.tile([CHUNK, 1], F32, tag="cs")
                nc.vector.tensor_tensor(cs, cnt, s0, op=ALU.subtract)
                wj5 = tn_pool.tile([CHUNK, KSHORT], F32, tag="wj5")
                nc.vector.tensor_scalar(wj5, iotk_sb, cs, None, op0=ALU.is_lt)
                gj = sm_pool.tile([CHUNK, D], F32, tag="gj")
                for j in range(KSHORT):
                    nc.vector.tensor_scalar_mul(gj, gath[:, j, :],
                                                wj5[:, j:j + 1])
                    if j == 0:
                        nc.vector.tensor_copy(sacc, gj)
                    else:
                        nc.vector.tensor_add(sacc, sacc, gj)
                short_sb = sm_pool.tile([CHUNK, D], F32, tag="short")
                nc.vector.tensor_scalar_mul(short_sb, sacc, drec)

                # ---- window means of pop/age -> gate ----
                lkg = sm_pool.tile([CHUNK, T], F32, tag="lkg")
                nc.vector.tensor_scalar(lkg, iot_sb, s0, None, op0=ALU.is_ge)
                lkl = sm_pool.tile([CHUNK, T], F32, tag="lkl")
                nc.vector.tensor_scalar(lkl, iot_sb, cnt, None, op0=ALU.is_lt)
                lk = sm_pool.tile([CHUNK, T], F32, tag="lk")
                nc.vector.tensor_mul(lk, lkg, lkl)
                lp = sm_pool.tile([CHUNK, T], F32, tag="lp")
                nc.vector.tensor_mul(lp, lk, pop_f)
                mp = tn_pool.tile([CHUNK, 1], F32, tag="mp")
                nc.vector.reduce_sum(mp, lp, axis=AX.X)
                nc.vector.tensor_mul(lp, lk, age_f)
                mr = tn_pool.tile([CHUNK, 1], F32, tag="mr")
                nc.vector.reduce_sum(mr, lp, axis=AX.X)
                z1 = tn_pool.tile([CHUNK, 1], F32, tag="z1")
                nc.vector.tensor_scalar_mul(z1, mp, gw0)
                z2 = tn_pool.tile([CHUNK, 1], F32, tag="z2")
                nc.vector.tensor_scalar_mul(z2, mr, gw1)
                nc.vector.tensor_add(z1, z1, z2)
                nc.vector.tensor_scalar_mul(z1, z1, drec)
                nc.vector.tensor_scalar_add(z1, z1, gb)
                ez = tn_pool.tile([CHUNK, 1], F32, tag="ez")
                nc.scalar.activation(ez, z1, ACT.Exp, bias=zero_c, scale=-1.0)
                ez1 = tn_pool.tile([CHUNK, 1], F32, tag="ez1")
                nc.vector.tensor_scalar_add(ez1, ez, 1.0)
                g = tn_pool.tile([CHUNK, 1], F32, tag="g")
                nc.vector.reciprocal(g, ez1)
                omg = tn_pool.tile([CHUNK, 1], F32, tag="omg")
                nc.vector.tensor_mul(omg, ez, g)

                # ---- combine + layernorm ----
                user = sm_pool.tile([CHUNK, D], F32, tag="user")
                nc.vector.tensor_scalar_mul(user, short_sb, g)
                ulong = sm_pool.tile([CHUNK, D], F32, tag="ulong")
                nc.vector.tensor_scalar_mul(ulong, long_sb, omg)
                nc.vector.tensor_add(user, user, ulong)

                stats = tn_pool.tile([CHUNK, 6], F32, tag="stats")
                nc.vector.bn_stats(stats, user)
                mv = tn_pool.tile([CHUNK, 2], F32, tag="mv")
                nc.vector.bn_aggr(mv, stats)
                veps = tn_pool.tile([CHUNK, 1], F32, tag="veps")
                nc.vector.tensor_scalar_add(veps, mv[:, 1:2], 1e-5)
                vrec = tn_pool.tile([CHUNK, 1], F32, tag="vrec")
                nc.vector.reciprocal(vrec, veps)  # 1/(var+eps)
                lnv = tn_pool.tile([CHUNK, 1], F32, tag="lnv")
                nc.scalar.activation(lnv, vrec, ACT.Ln, bias=tiny_c)
                rstd = tn_pool.tile([CHUNK, 1], F32, tag="rstd")
                nc.scalar.activation(rstd, lnv, ACT.Exp, bias=zero_c, scale=0.5)
                negmur = tn_pool.tile([CHUNK, 1], F32, tag="negmur")
                nc.vector.tensor_scalar(negmur, mv[:, 0:1], -1.0, rstd,
                                        op0=ALU.mult, op1=ALU.mult)
                usern = sm_pool.tile([CHUNK, D], F32, tag="usern")
                nc.scalar.activation(usern, user, ACT.Identity, bias=negmur,
                                     scale=rstd)
                nc.vector.tensor_mul(usern, usern, gam_sb)
                ou = sm_pool.tile([CHUNK, D], F32, tag="ou")
                nc.vector.tensor_add(ou, usern, bet_sb)
                nc.sync.dma_start(out=out[b0:b0 + CHUNK, :], in_=ou)

    nc.finalize()
    return nc


def _get_nc(alpha, gw0, gw1, gb):
    key = (round(alpha, 10), round(gw0, 10), round(gw1, 10), round(gb, 10))
    if key not in _CACHE:
        _CACHE[key] = _build(alpha, gw0, gw1, gb)
    return _CACHE[key]


def _run(inputs, trace=False):
    hist = np.ascontiguousarray(inputs["hist_items"], dtype=np.float32)
    mask = np.ascontiguousarray(inputs["hist_mask"]).astype(np.uint8)
    age = np.ascontiguousarray(inputs["hist_age_hours"], dtype=np.float32)
    pop = np.ascontiguousarray(inputs["hist_popularity"], dtype=np.float32)
    Wq = np.asarray(inputs["Wq"], dtype=np.float32)
    Wk = np.asarray(inputs["Wk"], dtype=np.float32)
    Wv = np.asarray(inputs["Wv"], dtype=np.float32)
    gate_w = np.asarray(inputs["gate_w"], dtype=np.float32)
    gate_b = np.asarray(inputs["gate_b"], dtype=np.float32)
    ln_gamma = np.asarray(inputs["ln_gamma"], dtype=np.float32)
    ln_beta = np.asarray(inputs["ln_beta"], dtype=np.float32)
    decay_alpha = float(np.asarray(inputs["decay_alpha"]))

    alpha = _softplus(decay_alpha) + 1e-6
    gw0, gw1 = float(gate_w[0, 0]), float(gate_w[0, 1])
    gb = float(gate_b[0])
    w2 = (Wq.T @ Wk) / np.sqrt(D)
    wvt = np.ascontiguousarray(Wv.T)
    gam = np.broadcast_to(ln_gamma, (CHUNK, D)).copy()
    bet = np.broadcast_to(ln_beta, (CHUNK, D)).copy()
    iot = np.broadcast_to(np.arange(T, dtype=np.float32), (CHUNK, T)).copy()
    rowb = np.empty((CHUNK, NCHUNK), np.float32)
    for c in range(NCHUNK):
        rowb[:, c] = (c * CHUNK + np.arange(CHUNK)) * T
    iotk = np.broadcast_to(np.arange(KSHORT, dtype=np.float32),
                           (CHUNK, KSHORT)).copy()

    nc = _get_nc(alpha, gw0, gw1, gb)
    in_maps = []
    for i in range(NCORES):
        sl = slice(i * BL, (i + 1) * BL)
        in_maps.append({
            "hist": hist[sl], "mask": mask[sl], "age": age[sl], "pop": pop[sl],
            "w2": w2, "wvt": wvt, "gam": gam, "bet": bet, "iot": iot,
            "rowb": rowb, "iotk": iotk,
        })
    res = run_bass_kernel_spmd(nc, in_maps, core_ids=list(range(NCORES)),
                               trace=trace)
    outs = [res.results[i]["out"] for i in range(NCORES)]
    full = np.concatenate(outs, axis=0).astype(np.float32)
    return full, res


def kernel(**inputs):
    return _run(inputs)[0]
